# revision 1
# baseline (speedup 1.0000x reference)
import sys

import numpy as np

for p in ("/opt/trn_rl_repo",):
    if p not in sys.path:
        sys.path.insert(0, p)

import concourse.bass as bass
import concourse.mybir as mybir
from concourse import bass_utils

N = 100000
E = 1600000
DIN = 256
HID = 64
DOUT = 64
NCORES = 8
PER = N // NCORES          # 12500 rows per core
MT = 512                   # moving free dim per matmul
NSLICE = 25                # ceil(12500/512) = 25
PAD = NSLICE * MT          # 12800 padded rows per core

_nc_cache = None


def _build_matmul_nc():
    """Per-core kernel: yt[64, PAD] = (w[256,64]).T @ xt[256, PAD] == (X @ W).T
    xt is the row-shard of X transposed; contraction dim 256 split into two
    128-partition chunks accumulated in PSUM."""
    nc = bass.Bass(target_bir_lowering=False)
    f32 = mybir.dt.float32

    xt = nc.dram_tensor("xt", [DIN, PAD], f32, kind="ExternalInput")
    w = nc.dram_tensor("w", [DIN, HID], f32, kind="ExternalInput")
    yt = nc.dram_tensor("yt", [HID, PAD], f32, kind="ExternalOutput")

    with (
        nc.semaphore("dma_sem") as dma_sem,
        nc.semaphore("mm_sem") as mm_sem,
        nc.semaphore("cp_sem") as cp_sem,
        nc.sbuf_tensor("xt0", [128, PAD], f32) as xt0,
        nc.sbuf_tensor("xt1", [128, PAD], f32) as xt1,
        nc.sbuf_tensor("w0", [128, HID], f32) as w0,
        nc.sbuf_tensor("w1", [128, HID], f32) as w1,
        nc.sbuf_tensor("ysb", [HID, PAD], f32) as ysb,
        nc.psum_tensor("acc0", [HID, MT], f32) as acc0,
        nc.psum_tensor("acc1", [HID, MT], f32) as acc1,
    ):
        accs = [acc0, acc1]
        with nc.Block() as block:

            @block.gpsimd
            def _(gpsimd):
                gpsimd.dma_start(xt0[:, :], xt[0:128, :]).then_inc(dma_sem, 16)
                gpsimd.dma_start(xt1[:, :], xt[128:256, :]).then_inc(dma_sem, 16)
                gpsimd.dma_start(w0[:, :], w[0:128, :]).then_inc(dma_sem, 16)
                gpsimd.dma_start(w1[:, :], w[128:256, :]).then_inc(dma_sem, 16)
                gpsimd.wait_ge(cp_sem, NSLICE)
                gpsimd.dma_start(yt[:, :], ysb[:, :]).then_inc(dma_sem, 16)
                gpsimd.wait_ge(dma_sem, 5 * 16)

            @block.tensor
            def _(tensor):
                tensor.wait_ge(dma_sem, 4 * 16)
                for m in range(NSLICE):
                    if m >= 2:
                        # psum double-buffer backpressure: copy of slice m-2 done
                        tensor.wait_ge(cp_sem, m - 1)
                    a = accs[m % 2]
                    lo, hi = m * MT, (m + 1) * MT
                    tensor.matmul(a[:, :], w0[:, :], xt0[:, lo:hi],
                                  start=True, stop=False)
                    tensor.matmul(a[:, :], w1[:, :], xt1[:, lo:hi],
                                  start=False, stop=True).then_inc(mm_sem, 1)

            @block.scalar
            def _(scalar):
                for m in range(NSLICE):
                    scalar.wait_ge(mm_sem, m + 1)
                    a = accs[m % 2]
                    lo, hi = m * MT, (m + 1) * MT
                    scalar.copy(ysb[:, lo:hi], a[:, :]).then_inc(cp_sem, 1)

    return nc


def _dev_matmul(X, W):
    """X [N,256] @ W [256,64] -> [N,64], sharded row-wise over 8 cores."""
    global _nc_cache
    if _nc_cache is None:
        _nc_cache = _build_matmul_nc()
    W = np.ascontiguousarray(W, dtype=np.float32)
    in_maps = []
    for c in range(NCORES):
        xs = X[c * PER:(c + 1) * PER]            # [12500, 256]
        xt = np.zeros((DIN, PAD), dtype=np.float32)
        xt[:, :PER] = xs.T
        in_maps.append({"xt": xt, "w": W})
    res = bass_utils.run_bass_kernel_spmd(_nc_cache, in_maps,
                                          core_ids=list(range(NCORES)))
    outs = res.results if hasattr(res, "results") else res
    parts = []
    for c in range(NCORES):
        r = outs[c]
        ytc = r["yt"] if isinstance(r, dict) else r
        parts.append(np.asarray(ytc)[:, :PER].T)
    return np.concatenate(parts, axis=0)         # [N, 64]


def kernel(x, edge_index, edge_weight, W1, b1, W2, b2):
    x = np.asarray(x, dtype=np.float32)
    ei = np.asarray(edge_index)
    ew = np.asarray(edge_weight, dtype=np.float32)
    W1 = np.asarray(W1, dtype=np.float32)
    b1 = np.asarray(b1, dtype=np.float32)
    W2 = np.asarray(W2, dtype=np.float32)
    b2 = np.asarray(b2, dtype=np.float32)
    src = ei[0].astype(np.int64)
    dst = ei[1].astype(np.int64)

    # degree incl. self-loop weight 1; symmetric normalization
    deg = np.zeros(N, dtype=np.float64)
    np.add.at(deg, dst, ew.astype(np.float64))
    deg += 1.0
    dinv = (1.0 / np.sqrt(deg)).astype(np.float32)
    norm_e = dinv[src] * ew * dinv[dst]          # [E]
    norm_self = dinv * dinv                      # [N]

    try:
        import scipy.sparse as sp
        A = sp.csr_matrix((norm_e, (dst, src)), shape=(N, N), dtype=np.float32)

        def agg(P):
            return A @ P + norm_self[:, None] * P
    except Exception:
        order = np.argsort(dst, kind="stable")
        s_s, d_s, n_s = src[order], dst[order], norm_e[order]

        def agg(P):
            out = norm_self[:, None] * P
            np.add.at(out, d_s, P[s_s] * n_s[:, None])
            return out

    # layer 1: relu(Â (x @ W1) + b1)
    P1 = _dev_matmul(x, W1)                      # [N, 64]
    h = np.maximum(agg(P1) + b1, 0.0).astype(np.float32)

    # layer 2: Â (h @ W2) + b2  (pad to reuse the same compiled kernel)
    hp = np.zeros((N, DIN), dtype=np.float32)
    hp[:, :HID] = h
    W2p = np.zeros((DIN, HID), dtype=np.float32)
    W2p[:HID, :] = W2
    P2 = _dev_matmul(hp, W2p)
    out = agg(P2) + b2
    return out.astype(np.float32)



# revision 2
# speedup vs baseline: 1.3786x; 1.3786x over previous
"""GCN 2-layer (GCNConv x2, relu between) on 8 trn2 NeuronCores, one launch.

Sharding: nodes row-sharded 12500/core (padded 12544). Edges bucketed by
(dst core, src core); per bucket, edges of one dst get occurrence ranks;
round r of a bucket has unique dsts, so its dma_scatter_add call is
race-free. Rounds r and r+4 share one of 4 accumulators; waves of 4 rounds
are serialized. Self-loop + symmetric normalization factorizes for ew == 1:
P' = dinv * (X @ W); acc_init = P'; out = dinv * sum(acc) + b.

Device pipeline per core: bf16 matmul (PE) -> dinv scale (DVE) -> HBM
AllGather of P' -> per-src-core dma_gather -> round scatter-adds -> merge ->
bias/relu -> layer 2 -> bf16 out. Falls back to a scipy host path for
pathological inputs (non-unit edge_weight or round-cap overflow).
"""
import sys
from contextlib import ExitStack

import numpy as np

for _p in ("/opt/trn_rl_repo",):
    if _p not in sys.path:
        sys.path.insert(0, _p)

import concourse.bass as bass
import concourse.mybir as mybir
from concourse import bass_utils
from concourse.library_config import mlp
from concourse.library_overlay import lower_extended_insts

f32 = mybir.dt.float32
bf16 = mybir.dt.bfloat16
i16 = mybir.dt.int16

NC = 8
N = 100000
DIN = 256
F = 64
CH = 512


class _Cfg:
    def __init__(self, n, caps):
        self.N = n
        self.PER = n // NC
        self.PERP = ((self.PER + 127) // 128) * 128
        self.NT = self.PERP // 128
        self.CAPS = list(caps)
        self.OFF = np.concatenate([[0], np.cumsum(caps)]).astype(np.int64)
        self.GTOK = int(self.OFF[-1])
        self.GBLK = self.GTOK // 128
        self.ABLK = (self.GBLK + 1) // 2
        self.SPLIT = self.ABLK * 128
        self.BBLK = self.GBLK - self.ABLK
        self.TOK = NC * self.GTOK
        self.ICOL = self.TOK // 16
        self.GCOL = self.GTOK // 16
        GMAX = 8064         # gather sub-call cap (ring + Q7 scratch)
        self.GSUBS = []     # (tok0, tok1, buf)
        for lo, hi, buf in ((0, self.SPLIT, 0), (self.SPLIT, self.GTOK, 1)):
            t = lo
            while t < hi:
                m = min(hi, t + GMAX)
                self.GSUBS.append((t, m, buf))
                t = m
        self.NGA = sum(1 for *_r, b in self.GSUBS if b == 0)
        self.NGB = len(self.GSUBS) - self.NGA
        SCMAX = 8064        # swdge ring: <=1024 tx descs per call
        self.SEGS = []
        for r in range(len(caps)):
            a, b = int(self.OFF[r]), int(self.OFF[r + 1])
            for lo, hi, buf in ((0, self.SPLIT, 0), (self.SPLIT, self.GTOK, 1)):
                s, e = max(a, lo), min(b, hi)
                while s < e:
                    m = min(e, s + SCMAX)
                    self.SEGS.append((s, m, buf, r // 4, r % 4))
                    s = m
        self.NSC = len(self.SEGS)
        self.PAD_ROW = self.PER
        self.NCH = (self.PERP + CH - 1) // CH


CFG = _Cfg(N, [11392, 7936, 4608, 2176, 1024, 512, 256, 128, 128, 128,
               128, 128])


def _build_nc(cfg, num_devices=NC):
    PERP, NT, NCH, NSC = cfg.PERP, cfg.NT, cfg.NCH, cfg.NSC
    nc = bass.Bass(target_bir_lowering=False, num_devices=num_devices,
                   num_swdge_queues=4)

    xt = nc.dram_tensor("xt", [2, 128, PERP], bf16, kind="ExternalInput")
    w1 = nc.dram_tensor("w1", [2, 128, F], bf16, kind="ExternalInput")
    w2 = nc.dram_tensor("w2", [64, F], bf16, kind="ExternalInput")
    b1r = nc.dram_tensor("b1r", [128, F], f32, kind="ExternalInput")
    b2r = nc.dram_tensor("b2r", [128, F], f32, kind="ExternalInput")
    dinvt = nc.dram_tensor("dinvt", [128, NT], f32, kind="ExternalInput")
    gidx = nc.dram_tensor("gidx", [16, cfg.ICOL], i16, kind="ExternalInput")
    sidx = nc.dram_tensor("sidx", [16, cfg.ICOL], i16, kind="ExternalInput")
    identb = nc.dram_tensor("identb", [128, 128], bf16, kind="ExternalInput")
    identf = nc.dram_tensor("identf", [128, 128], f32, kind="ExternalInput")
    out = nc.dram_tensor("out", [PERP, F], bf16, kind="ExternalOutput")

    cc_in = nc.dram_tensor("cc_in", [PERP, F], f32)
    cc_out = nc.dram_tensor("cc_out", [NC * PERP, F], f32)
    acc = nc.dram_tensor("acc", [4, PERP, F], f32)

    NSC1 = (NSC + 1) // 2
    NSC2 = NSC // 2

    with ExitStack() as ctx:
        sems = ["s_in", "s_idx", "s_xt0", "s_xt1", "s_mmc", "s_p1c", "s_ptr", "s_nbc",
                "s_tr", "s_htc", "s_mm", "s_nbw", "s_sc", "s_mg", "s_ccin",
                "s_acc", "s_cc", "s_g0", "s_g3", "s_s1", "s_s2", "s_mrg", "s_a3", "s_hrdy",
                "s_ob", "s_fin", "s_v"]
        S = {n: ctx.enter_context(nc.semaphore(n)) for n in sems}

        xt_sb = [ctx.enter_context(nc.sbuf_tensor(f"xts{i}", [128, CH], bf16))
                 for i in range(4)]
        w1_sb = ctx.enter_context(nc.sbuf_tensor("w1s", [128, 2, F], bf16))
        w2_sb = ctx.enter_context(nc.sbuf_tensor("w2s", [64, F], bf16))
        b_sb = ctx.enter_context(nc.sbuf_tensor("bs", [128, 2, F], f32))
        dinv_sb = ctx.enter_context(nc.sbuf_tensor("dinvs", [128, NT], f32))
        idb_sb = ctx.enter_context(nc.sbuf_tensor("idb", [128, 128], bf16))
        idf_sb = ctx.enter_context(nc.sbuf_tensor("idf", [128, 128], f32))
        gidx_sb = ctx.enter_context(nc.sbuf_tensor("gix", [128, cfg.ICOL], i16))
        sidx_sb = ctx.enter_context(nc.sbuf_tensor("six", [128, cfg.ICOL], i16))
        p1t_sb = ctx.enter_context(nc.sbuf_tensor("p1t", [64, PERP], bf16))
        nodebuf = ctx.enter_context(nc.sbuf_tensor("nb", [128, NT, F], f32))
        msgA = ctx.enter_context(nc.sbuf_tensor("mA", [128, cfg.ABLK, F], f32))
        msgB = ctx.enter_context(nc.sbuf_tensor("mB", [128, cfg.BBLK, F], f32))
        obuf = ctx.enter_context(nc.sbuf_tensor("ob", [128, NT, F], bf16))
        hT_sb = [ctx.enter_context(nc.sbuf_tensor(f"hT{i}", [64, 128], bf16))
                 for i in range(2)]

        ps1 = [ctx.enter_context(nc.psum_tensor(f"ps1{i}", [64, CH], f32))
               for i in range(2)]
        psT = [ctx.enter_context(nc.psum_tensor(f"psT{i}", [128, F], bf16))
               for i in range(2)]
        psH = [ctx.enter_context(nc.psum_tensor(f"psH{i}", [64, 128], f32))
               for i in range(2)]
        psM = [ctx.enter_context(nc.psum_tensor(f"psM{i}", [128, F], f32))
               for i in range(2)]

        dinv_b = dinv_sb[:, :].unsqueeze(2).broadcast_to([128, NT, F])

        def ccv(t2d):
            return t2d.rearrange("(t p) f -> p t f", p=128)

        def chunks_needed(ti):
            return (128 * (ti + 1) + CH - 1) // CH

        with nc.Block() as block:

            @block.sync
            def _(s):
                for ap_d, ap_s in [
                        (w1_sb[:, :, :], w1[:, :, :].rearrange("k p f -> p k f")),
                        (w2_sb[:, :], w2[:, :]),
                        (b_sb[:, 0, :], b1r[:, :]),
                        (b_sb[:, 1, :], b2r[:, :]),
                        (dinv_sb[:, :], dinvt[:, :]),
                        (idb_sb[:, :], identb[:, :]),
                        (idf_sb[:, :], identf[:, :])]:
                    s.dma_start(ap_d, ap_s).then_inc(S["s_in"], 16)
                for r in range(8):
                    s.dma_start(gidx_sb[16 * r:16 * (r + 1), :],
                                gidx[:, :]).then_inc(S["s_in"], 16)
                    s.dma_start(sidx_sb[16 * r:16 * (r + 1), :],
                                sidx[:, :]).then_inc(S["s_in"], 16)

                for c in range(NCH):
                    lo = c * CH
                    hi = min(PERP, lo + CH)
                    bi = c % 2
                    if c >= 2:
                        s.wait_ge(S["s_mmc"], c - 1)
                    sx = S["s_xt0"] if bi == 0 else S["s_xt1"]
                    s.dma_start(xt_sb[2 * bi][:, 0:hi - lo],
                                xt[0, :, lo:hi]).then_inc(sx, 16)
                    s.dma_start(xt_sb[2 * bi + 1][:, 0:hi - lo],
                                xt[1, :, lo:hi]).then_inc(sx, 16)

                for L in range(2):
                    s.wait_ge(S["s_sc"], L + 1)
                    s.dma_start(ccv(cc_in[:, :]), nodebuf[:, :, :]).then_inc(
                        S["s_ccin"], 16)
                    s.dma_start(ccv(acc[0, :, :]), nodebuf[:, :, :]).then_inc(
                        S["s_acc"], 16)
                    s.wait_ge(S["s_mg"], L + 1)
                    for a in range(1, 4):
                        s.dma_start(ccv(acc[a, :, :]),
                                    msgA[:, 0:NT, :]).then_inc(S["s_acc"], 16)
                    s.wait_ge(S["s_s1"], 16 * NSC1 * NC * (L + 1))
                    s.wait_ge(S["s_s2"], 16 * NSC2 * NC * (L + 1))
                    s.dma_start(nodebuf[:, :, :], ccv(acc[0, :, :])).then_inc(
                        S["s_mrg"], 16)
                    s.dma_start(msgA[:, 0:NT, :], ccv(acc[1, :, :])).then_inc(
                        S["s_mrg"], 16)
                    s.dma_start(msgB[:, 0:NT, :], ccv(acc[2, :, :])).then_inc(
                        S["s_mrg"], 16)
                    s.wait_ge(S["s_a3"], L + 1)
                    s.dma_start(msgA[:, 0:NT, :], ccv(acc[3, :, :])).then_inc(
                        S["s_mrg"], 16)

                s.wait_ge(S["s_ob"], 1)
                s.dma_start(ccv(out[:, :]), obuf[:, :, :]).then_inc(
                    S["s_fin"], 16)
                s.wait_ge(S["s_fin"], 16)

            @block.vector
            def _(v):
                vt = 0
                for L in range(2):
                    if L == 0:
                        v.wait_ge(S["s_nbc"], NT)
                    else:
                        v.wait_ge(S["s_nbw"], NT)
                    v.tensor_tensor(nodebuf[:, :, :], nodebuf[:, :, :], dinv_b,
                                    mybir.AluOpType.mult).then_inc(S["s_sc"], 1)
                    v.memset(msgA[:, :, :], 0.0).then_inc(S["s_mg"], 1)

                    v.wait_ge(S["s_mrg"], 64 * L + 48)
                    v.tensor_add(nodebuf[:, :, :], nodebuf[:, :, :],
                                 msgA[:, 0:NT, :]).then_inc(S["s_a3"], 1)
                    v.wait_ge(S["s_a3"], L + 1)
                    v.tensor_add(nodebuf[:, :, :], nodebuf[:, :, :],
                                 msgB[:, 0:NT, :]).then_inc(S["s_v"], 1)
                    vt += 1
                    v.wait_ge(S["s_mrg"], 64 * L + 64)
                    v.wait_ge(S["s_v"], vt)
                    v.tensor_add(nodebuf[:, :, :], nodebuf[:, :, :],
                                 msgA[:, 0:NT, :]).then_inc(S["s_v"], 1)
                    vt += 1
                    v.wait_ge(S["s_v"], vt)
                    v.tensor_tensor(nodebuf[:, :, :], nodebuf[:, :, :], dinv_b,
                                    mybir.AluOpType.mult).then_inc(S["s_v"], 1)
                    vt += 1
                    bias_b = b_sb[:, L, :].unsqueeze(1).broadcast_to(
                        [128, NT, F])
                    v.wait_ge(S["s_v"], vt)
                    v.tensor_tensor(nodebuf[:, :, :], nodebuf[:, :, :], bias_b,
                                    mybir.AluOpType.add).then_inc(S["s_v"], 1)
                    vt += 1
                    v.wait_ge(S["s_v"], vt)
                    if L == 0:
                        v.tensor_relu(nodebuf[:, :, :],
                                      nodebuf[:, :, :]).then_inc(S["s_hrdy"], 1)
                    else:
                        v.tensor_copy(obuf[:, :, :],
                                      nodebuf[:, :, :]).then_inc(S["s_ob"], 1)

            @block.tensor
            def _(t):
                t.wait_ge(S["s_in"], 368)
                for c in range(NCH):
                    lo = c * CH
                    hi = min(PERP, lo + CH)
                    bi = c % 2
                    t.wait_ge(S["s_xt0"] if c % 2 == 0 else S["s_xt1"],
                              32 * (c // 2 + 1))
                    if c >= 2:
                        t.wait_ge(S["s_p1c"], c - 1)
                    t.matmul(ps1[bi][:, 0:hi - lo], w1_sb[:, 0, :],
                             xt_sb[2 * bi][:, 0:hi - lo], start=True,
                             stop=False)
                    t.matmul(ps1[bi][:, 0:hi - lo], w1_sb[:, 1, :],
                             xt_sb[2 * bi + 1][:, 0:hi - lo],
                             start=False, stop=True).then_inc(S["s_mmc"], 1)
                for ti in range(NT):
                    t.wait_ge(S["s_p1c"], chunks_needed(ti))
                    if ti >= 2:
                        t.wait_ge(S["s_nbc"], ti - 1)
                    t.transpose(psT[ti % 2][:, :],
                                p1t_sb[:, ti * 128:(ti + 1) * 128],
                                idb_sb[0:64, 0:64]).then_inc(S["s_ptr"], 1)

                t.wait_ge(S["s_hrdy"], 1)
                for ti in range(NT):
                    b = ti % 2
                    if ti >= 2:
                        t.wait_ge(S["s_htc"], ti - 1)
                    t.transpose(psH[b][:, :], nodebuf[:, ti, :],
                                idf_sb[:, :]).then_inc(S["s_tr"], 1)
                    t.wait_ge(S["s_htc"], ti + 1)
                    if ti >= 2:
                        t.wait_ge(S["s_nbw"], ti - 1)
                    t.matmul(psM[b][:, :], hT_sb[b][:, :], w2_sb[:, :],
                             start=True, stop=True).then_inc(S["s_mm"], 1)

            @block.scalar
            def _(sc):
                for c in range(NCH):
                    lo = c * CH
                    hi = min(PERP, lo + CH)
                    sc.wait_ge(S["s_mmc"], c + 1)
                    sc.copy(p1t_sb[:, lo:hi],
                            ps1[c % 2][:, 0:hi - lo]).then_inc(S["s_p1c"], 1)
                for ti in range(NT):
                    sc.wait_ge(S["s_ptr"], ti + 1)
                    sc.copy(nodebuf[:, ti, :], psT[ti % 2][:, :]).then_inc(
                        S["s_nbc"], 1)

                sc.wait_ge(S["s_hrdy"], 1)
                for ti in range(NT):
                    b = ti % 2
                    sc.wait_ge(S["s_tr"], ti + 1)
                    if ti >= 2:
                        sc.wait_ge(S["s_mm"], ti - 1)
                    sc.copy(hT_sb[b][:, :], psH[b][:, :]).then_inc(
                        S["s_htc"], 1)
                    sc.wait_ge(S["s_mm"], ti + 1)
                    sc.copy(nodebuf[:, ti, :], psM[b][:, :]).then_inc(
                        S["s_nbw"], 1)

            @block.gpsimd
            def _(g):
                g.load_library(mlp)
                g.wait_ge(S["s_in"], 368)
                sizes = {t1 - t0 for t0, t1, _b in cfg.GSUBS}
                sizes.update(s1 - s0 for s0, s1, _b, _w, _a in cfg.SEGS)
                reg = {v: g.to_reg(v) for v in sorted(sizes)}
                ns1 = ns2 = 0

                def wait_scat():
                    if ns1:
                        g.wait_ge(S["s_s1"], 16 * ns1)
                    if ns2:
                        g.wait_ge(S["s_s2"], 16 * ns2)

                for L in range(2):
                    g.wait_ge(S["s_ccin"], 16 * (L + 1))
                    g.collective_compute(
                        "AllGather", mybir.AluOpType.bypass,
                        replica_groups=[list(range(num_devices))],
                        ins=[cc_in[:, :].opt()],
                        outs=[cc_out[:, :].opt()],
                    ).then_inc(S["s_cc"], 1)
                    g.wait_ge(S["s_cc"], L + 1)
                    g.wait_ge(S["s_acc"], 64 * (L + 1))
                    for grp in range(NC):
                        base = grp * cfg.GCOL
                        wait_scat()
                        cyc = NC * L + grp
                        nga = ngb = 0
                        for t0, t1, buf in cfg.GSUBS:
                            mb = msgB if buf else msgA
                            off = cfg.SPLIT if buf else 0
                            ins = g.dma_gather(
                                mb[:, (t0 - off) // 128:(t1 - off) // 128, :],
                                cc_out[grp * PERP:(grp + 1) * PERP, :],
                                gidx_sb[:, base + t0 // 16:base + t1 // 16],
                                t1 - t0, reg[t1 - t0], F,
                                single_packet=False,
                                queue_num=3 if buf else 0)
                            if buf:
                                ngb += 1
                                ins.then_inc(S["s_g3"], 16)
                            else:
                                nga += 1
                                ins.then_inc(S["s_g0"], 16)
                        gat_a = 16 * cfg.NGA * (cyc + 1)
                        gat_b = 16 * cfg.NGB * (cyc + 1)
                        prev_wave = -1
                        for k, (s0, s1, buf, wave, a) in enumerate(cfg.SEGS):
                            if wave != prev_wave:
                                wait_scat()
                                prev_wave = wave
                            g.wait_ge(S["s_g3"] if buf else S["s_g0"],
                                      gat_b if buf else gat_a)
                            mb = msgB if buf else msgA
                            off = cfg.SPLIT if buf else 0
                            ins = g.dma_scatter_add(
                                acc[a, :, :],
                                mb[:, (s0 - off) // 128:(s1 - off) // 128, :],
                                sidx_sb[:, base + s0 // 16:base + s1 // 16],
                                s1 - s0, reg[s1 - s0], F,
                                queue_num=1 + k % 2)
                            if k % 2 == 0:
                                ins.then_inc(S["s_s1"], 16)
                                ns1 += 1
                            else:
                                ins.then_inc(S["s_s2"], 16)
                                ns2 += 1

    lower_extended_insts(nc)
    return nc


def _preprocess(cfg, src, dst):
    E = src.shape[0]
    PER = cfg.PER
    src = src.astype(np.int64)
    dst = dst.astype(np.int64)
    core = dst // PER
    grp = src // PER
    bucket = (core * 8 + grp).astype(np.int32)
    dloc = (dst - core * PER).astype(np.int32)
    sloc = (src - grp * PER).astype(np.int32)

    key = bucket * np.int32(PER) + dloc
    order = np.argsort(key, kind="stable")
    kb = key[order]
    sb = sloc[order]
    db = dloc[order]
    bb = bucket[order]

    flag = np.empty(E, bool)
    flag[0] = True
    np.not_equal(kb[1:], kb[:-1], out=flag[1:])
    run_starts = np.flatnonzero(flag)
    run_id = np.cumsum(flag) - 1
    occ = np.arange(E) - run_starts[run_id]
    ncalls = len(cfg.CAPS)
    if occ.max() >= ncalls:
        return None

    callk = bb.astype(np.int64) * ncalls + occ
    corder = np.argsort(callk, kind="stable")
    ck = callk[corder]
    cflag = np.empty(E, bool)
    cflag[0] = True
    np.not_equal(ck[1:], ck[:-1], out=cflag[1:])
    cstart = np.maximum.accumulate(np.where(cflag, np.arange(E), 0))
    pic = np.empty(E, np.int64)
    pic[corder] = np.arange(E) - cstart

    csizes = np.bincount(callk, minlength=64 * ncalls).reshape(64, ncalls)
    if (csizes > np.asarray(cfg.CAPS)).any():
        return None

    token = (bb % 8) * np.int64(cfg.GTOK) + cfg.OFF[occ] + pic
    core_b = bb // 8
    gidx = np.zeros((NC, cfg.TOK), np.int16)
    sidx = np.full((NC, cfg.TOK), cfg.PAD_ROW, np.int16)
    flat = core_b * np.int64(cfg.TOK) + token
    gidx.reshape(-1)[flat] = sb.astype(np.int16)
    sidx.reshape(-1)[flat] = db.astype(np.int16)
    return gidx, sidx


def _wrap16(tok):
    n = tok.shape[0]
    return tok.reshape(n // 16, 16).T.copy()


_nc_cache = None


def _run_device(cfg, x, src, dst, ew, W1, b1, W2, b2, pre):
    import ml_dtypes
    global _nc_cache
    gidx, sidx = pre
    deg = np.bincount(dst.astype(np.int64), weights=ew.astype(np.float64),
                      minlength=cfg.N) + 1.0
    dinv = (1.0 / np.sqrt(deg)).astype(np.float32)

    PER, PERP, NT = cfg.PER, cfg.PERP, cfg.NT
    dt = np.zeros((NC, PERP), np.float32)
    dt[:, :PER] = dinv.reshape(NC, PER)
    dinvt = np.ascontiguousarray(dt.reshape(NC, NT, 128).transpose(0, 2, 1))

    bf = ml_dtypes.bfloat16
    xb = np.asarray(x, np.float32).astype(bf)
    w1b = np.ascontiguousarray(np.asarray(W1, np.float32).astype(bf)
                               .reshape(2, 128, F))
    w2b = np.asarray(W2, np.float32).astype(bf)
    b1r = np.broadcast_to(np.asarray(b1, np.float32), (128, F)).copy()
    b2r = np.broadcast_to(np.asarray(b2, np.float32), (128, F)).copy()
    identb = np.eye(128, dtype=bf)
    identf = np.eye(128, dtype=np.float32)

    in_maps = []
    for c in range(NC):
        xs = np.zeros((2, 128, PERP), bf)
        xs[:, :, :PER] = xb[c * PER:(c + 1) * PER].T.reshape(2, 128, PER)
        in_maps.append({
            "xt": xs, "w1": w1b, "w2": w2b, "b1r": b1r, "b2r": b2r,
            "dinvt": dinvt[c],
            "gidx": _wrap16(gidx[c]), "sidx": _wrap16(sidx[c]),
            "identb": identb, "identf": identf,
        })

    if _nc_cache is None:
        _nc_cache = _build_nc(cfg)
    res = bass_utils.run_bass_kernel_spmd(_nc_cache, in_maps,
                                          core_ids=list(range(NC)))
    outs = res.results if hasattr(res, "results") else res
    parts = [np.asarray(outs[c]["out"][:PER], np.float32) for c in range(NC)]
    return np.concatenate(parts, axis=0)


def _fallback(x, src, dst, ew, W1, b1, W2, b2):
    import scipy.sparse as sp
    x = np.asarray(x, np.float32)
    deg = np.bincount(dst, weights=ew.astype(np.float64), minlength=N) + 1.0
    dinv = (1.0 / np.sqrt(deg)).astype(np.float32)
    nv = dinv[src] * ew.astype(np.float32) * dinv[dst]
    A = sp.csr_matrix((nv, (dst, src)), shape=(N, N), dtype=np.float32)
    ns = dinv * dinv

    def agg(P):
        return A @ P + ns[:, None] * P

    h = np.maximum(agg(x @ W1) + b1, 0.0).astype(np.float32)
    return (agg(h @ W2) + b2).astype(np.float32)


def kernel(x, edge_index, edge_weight, W1, b1, W2, b2):
    x = np.asarray(x, np.float32)
    ei = np.asarray(edge_index)
    ew = np.asarray(edge_weight, np.float32)
    W1 = np.asarray(W1, np.float32)
    b1 = np.asarray(b1, np.float32)
    W2 = np.asarray(W2, np.float32)
    b2 = np.asarray(b2, np.float32)
    src = ei[0].astype(np.int64)
    dst = ei[1].astype(np.int64)

    if x.shape != (N, DIN) or not np.all(ew == 1.0):
        return _fallback(x, src, dst, ew, W1, b1, W2, b2)
    pre = _preprocess(CFG, src, dst)
    if pre is None:
        return _fallback(x, src, dst, ew, W1, b1, W2, b2)
    return _run_device(CFG, x, src, dst, ew, W1, b1, W2, b2, pre)
